# revision 22
# baseline (speedup 1.0000x reference)
"""Trainium2 Bass kernel for hash-indexed gather + GEMM (gnn_message_passing).

Reference computation:
    g[b, c, hw, k] = x.reshape(B, C*H*W)[b, hash_idx[c, hw, k]]
    out[kp, b*hw]  = weights[kp, c*k] @ g  (contraction over ck = 288)
    returns [B, KP, H, W]

Strategy (8 NeuronCores, no cross-core communication):
  - Host transposes x to xT[C*H*W, B] float32 so one gathered "row" is the
    value of one (c,pixel) across all 64 images = 256 contiguous bytes.
  - Each core owns 128 of the 1024 output pixels (all images, all channels).
  - On-device dma_gather (SWDGE) gathers rows straight from HBM and writes
    row i to SBUF partition i%128; we order the index list so partitions
    line up with the GEMM contraction dim (ck), i.e. the gather performs the
    im2col transpose for free.
  - 3 ck-chunks (0:128, 128:256, 256:288) accumulate into PSUM. The 32-wide
    third chunk is packed 4x along partitions covering hw%4 classes.
  - Output per core: [KP=64, 8192] = (m4, q, b) column order, reassembled on
    host.
"""

import numpy as np

B, C, H, W = 64, 32, 32, 32
K = 9
KP = 64
HWT = H * W          # 1024
CHW = C * H * W      # 32768
CK = C * K           # 288
NCORES = 8
HWC = HWT // NCORES  # 128 pixels per core
NCOLS = HWC * B      # 8192 output columns per core

_NC_CACHE = {}


def _build_nc():
    """Build the (single-program SPMD) Bass graph; all per-core variation is
    carried by the input data."""
    from concourse import bacc, bass, library_config, mybir, tile

    f32 = mybir.dt.float32
    f32r = mybir.dt.float32r
    i16 = mybir.dt.int16

    nc = bacc.Bacc(num_swdge_queues=4)

    xt = nc.declare_dram_parameter("xt", [CHW, B], f32, isOutput=False)
    idx = nc.declare_dram_parameter("idx", [128, 2304], i16, isOutput=False)
    w = nc.declare_dram_parameter("w", [128, 256], f32, isOutput=False)
    out = nc.declare_dram_parameter("out", [KP, NCOLS], f32, isOutput=True)

    # Load the GPSIMD library before the TileContext preamble so the ~10us
    # IRAM load overlaps the framework's start-of-block barriers.
    nc.gpsimd.load_library(library_config.mlp)

    with tile.TileContext(nc) as tc:
        with (
            tc.tile_pool(name="const", bufs=1) as const_pool,
            tc.tile_pool(name="g", bufs=1) as g_pool,
            tc.tile_pool(name="stage", bufs=1) as stage_pool,
            tc.tile_pool(name="psum", bufs=6, space="PSUM") as psum_pool,
        ):

            idx_sb = const_pool.tile([128, 2304], i16)
            nc.sync.dma_start(out=idx_sb[:], in_=idx[:])
            # float32r tiles: same bits as f32, but lets the fp32r matmuls
            # (1 cycle/row vs 4 for fp32) pass BIR verification.
            w_sb = const_pool.tile([128, 256], f32r)
            nc.sync.dma_start(out=w_sb[:], in_=w[:].bitcast(f32r))

            # Gather targets as per-subcall tiles so Tile's dependency
            # tracking lets supertile s start as soon as ITS slices landed.
            # 2048-idx calls (16 slots / 2 supertiles each) amortize the
            # ~550ns fixed SWDGE cost; the tail calls are 1024-idx so the
            # last round finishes sooner. Schedule: 5 calls per queue
            # (4x2048 + 1x1024 = 9216 rows), one queue per GPSIMD cpu pair.
            # 2048-row calls (16 slots / 2 supertiles each) amortize the
            # ~550ns fixed SWDGE cost; 1024-row tail calls shorten the final
            # round. Per queue: [2048 x4, 1024] = 9216 rows; the four SWDGE
            # queues map to the four GPSIMD cpu pairs and stream ~4-way.
            g0p = [g_pool.tile([128, 16, B], f32r, tag=f"g0p_{t}",
                               name=f"g0p_{t}") for t in range(7)]
            g1p = [g_pool.tile([128, 16, B], f32r, tag=f"g1p_{t}",
                               name=f"g1p_{t}") for t in range(7)]
            g2p = [g_pool.tile([128, 16, B], f32r, tag=f"g2p_{v}",
                               name=f"g2p_{v}") for v in range(2)]
            g0h = [g_pool.tile([128, 8, B], f32r, tag=f"g0h_{h}",
                               name=f"g0h_{h}") for h in range(2)]
            g1h = [g_pool.tile([128, 8, B], f32r, tag=f"g1h_{h}",
                               name=f"g1h_{h}") for h in range(2)]

            nidx_regs = {1024: nc.gpsimd.to_reg(1024),
                         2048: nc.gpsimd.to_reg(2048)}

            def gather(dst, col0, q, nidx=2048):
                nc.gpsimd.dma_gather(
                    dst[:], xt[:].bitcast(f32r),
                    idx_sb[:, col0:col0 + nidx // 16], nidx,
                    nidx_regs[nidx], B, queue_num=q, single_packet=False,
                )

            # Round 1 unblocks supertiles 0..3 (g2 first: shared by all
            # s with matching s%4); rounds 2-4 stream g0/g1 pairs; round 5
            # is the 1024-row tail for supertiles 14/15.
            gather(g2p[0], 2048 + 0, 2)
            gather(g2p[1], 2048 + 128, 3)
            gather(g0p[0], 0, 0)
            gather(g1p[0], 1024, 1)
            for rnd in range(3):
                ta, tb = 1 + 2 * rnd, 2 + 2 * rnd
                gather(g0p[ta], 128 * ta, 0)
                gather(g1p[ta], 1024 + 128 * ta, 1)
                gather(g0p[tb], 128 * tb, 2)
                gather(g1p[tb], 1024 + 128 * tb, 3)
            gather(g0h[0], 896, 0, nidx=1024)
            gather(g1h[0], 1024 + 896, 1, nidx=1024)
            gather(g0h[1], 960, 2, nidx=1024)
            gather(g1h[1], 1024 + 960, 3, nidx=1024)

            stage_t = [stage_pool.tile([KP, 1024], f32, tag=f"st_{r}",
                                       name=f"st_{r}")
                       for r in range(8)]

            def mm(ps, stat, mov, start, stop):
                # float32r: full-rate (1 cycle/row) fp32 matmul on trn2.
                nc.tensor.matmul(ps, stat, mov, start=start, stop=stop)

            def mm_chunk2(ps, m4, g2tile, s0, s1):
                # chunk2: partitions 32*m4..+32 of g2 (AP base must be
                # 0/32/64; the base-96 block runs as K=64 at base 64 with
                # zeroed weight rows for partitions 64..95).
                if m4 < 3:
                    mm(ps, w_sb[32 * m4:32 * m4 + 32, 128:192],
                       g2tile[32 * m4:32 * m4 + 32, s0:s1, :],
                       start=False, stop=True)
                else:
                    mm(ps, w_sb[64:128, 192:256],
                       g2tile[64:128, s0:s1, :], start=False, stop=True)

            # 16 supertiles of 512 output columns; supertile s: m4 = s//4
            # (hw%4 class), q-range 8*(s%4)..+8 (hw//4). s consumes half of
            # gather tile s//2 (supertiles 14/15 have their own 1024-idx
            # tail tiles).
            for s in range(16):
                if s < 14:
                    o = 8 * (s % 2)
                    ga = g0p[s // 2][:, o:o + 8, :]
                    gb = g1p[s // 2][:, o:o + 8, :]
                else:
                    ga = g0h[s - 14][:]
                    gb = g1h[s - 14][:]
                m4, u = s // 4, s % 4
                o2 = 8 * (u % 2)
                ps = psum_pool.tile([KP, 512], f32)
                mm(ps[:], w_sb[:, 0:64], ga, start=True, stop=False)
                mm(ps[:], w_sb[:, 64:128], gb, start=False, stop=False)
                mm_chunk2(ps[:], m4, g2p[u // 2], o2, o2 + 8)
                eng = nc.vector.tensor_copy if s % 2 == 0 else nc.scalar.copy
                eng(out=stage_t[s // 2][:, 512 * (s % 2):512 * (s % 2 + 1)],
                    in_=ps[:])
                if s % 2 == 1:
                    nc.sync.dma_start(
                        out=out[:, 1024 * (s // 2):1024 * (s // 2 + 1)],
                        in_=stage_t[s // 2][:],
                    )

    nc.finalize()
    _hoist_preamble(nc)
    return nc


def _hoist_preamble(nc):
    """Move the ~11us GPSIMD library IRAM load to the very start of the Pool
    stream (before the framework init call + all-engine barrier), and the
    idx/w input DMAs into the entry block, so both overlap engine init
    instead of serializing after it."""
    from concourse import mybir

    entry = nc.main_func.blocks[0]
    il = entry.instructions
    reload_ = next(
        i for i in il if type(i).__name__ == "InstPseudoReloadLibraryIndex"
    )
    il.remove(reload_)
    il.insert(0, reload_)

    body = nc.main_func.blocks[1]
    bl = body.instructions
    dmas = [i for i in bl if isinstance(i, mybir.InstDMACopy)][:2]
    # the first two DMA copies are the idx_sb / w_sb input loads
    pos = 2  # right after [reload, InstCall]
    for d in dmas:
        bl.remove(d)
        il.insert(pos, d)
        pos += 1


def get_nc():
    if "nc" not in _NC_CACHE:
        _NC_CACHE["nc"] = _build_nc()
    return _NC_CACHE["nc"]


def _wrap_idx(flat):
    """SWDGE index layout: unwrapped[i] = wrapped[i % 16, i // 16], replicated
    across the 8 GPSIMD 16-partition groups."""
    n = flat.shape[0]
    wrapped = flat.reshape(n // 16, 16).T  # [16, n/16]
    return np.tile(wrapped, (8, 1))        # [128, n/16]


def make_inputs(x, hash_idx, weights):
    """Host-side sharding/layout prep. Returns in_maps for the 8 cores."""
    x = np.asarray(x, dtype=np.float32)
    hash_idx = np.asarray(hash_idx)
    weights = np.asarray(weights, dtype=np.float32)

    xtr = np.ascontiguousarray(x.reshape(B, CHW).T)  # [CHW, B] f32

    # idxmat[ck, hw] with ck = c*9 + k
    idxmat = (
        hash_idx.reshape(C, HWT, K).transpose(0, 2, 1).reshape(CK, HWT)
    ).astype(np.int16)

    # weights, stationary layout: [contraction partitions, 64 kp]
    w_sb = np.zeros((128, 256), dtype=np.float32)
    w_sb[:, 0:64] = weights[:, 0:128].T
    w_sb[:, 64:128] = weights[:, 128:256].T
    w_sb[:, 128:192] = np.tile(weights[:, 256:288].T, (4, 1))
    # m4=3 special stationary: rows 64..95 (m4=2 data) zero, 96..127 real
    w_sb[96:128, 192:256] = weights[:, 256:288].T

    # column order inside a core: col = m4*2048 + q*64 + b ; hw_local = 4q+m4
    j_arange = np.arange(HWC)
    hw_of_slot = 4 * (j_arange % 32) + j_arange // 32  # slot j -> hw_local

    in_maps = []
    for m in range(NCORES):
        sub = idxmat[:, m * HWC:(m + 1) * HWC]  # [288, 128] int16
        # chunks 0/1: i = j*128 + p (slot-major, ck-local minor)
        c0 = np.ascontiguousarray(sub[0:128, hw_of_slot].T).reshape(-1)
        c1 = np.ascontiguousarray(sub[128:256, hw_of_slot].T).reshape(-1)
        # chunk 2: i = q*128 + m4*32 + ck_local ; partition = 32*m4+ck
        #   value  = idx[256+ck, hw = 4q+m4]
        sub2 = sub[256:288, :]                     # [32, 128]
        arr2 = sub2.T.reshape(32, 4, 32)           # [q, m4, ck]
        c2 = np.ascontiguousarray(arr2).reshape(-1)
        idx_all = np.concatenate(
            [_wrap_idx(c0), _wrap_idx(c1), _wrap_idx(c2)], axis=1
        )
        idx_all = np.ascontiguousarray(idx_all, dtype=np.int16)
        in_maps.append({"xt": xtr, "idx": idx_all, "w": w_sb})
    return in_maps


def assemble_output(shards):
    """shards[m]: [KP, 8192] in (m4, q, b) column order -> [B, KP, H, W]."""
    out = np.empty((B, KP, HWT), dtype=np.float32)
    for m in range(NCORES):
        sh = np.asarray(shards[m]).reshape(KP, 4, 32, B)  # [kp, m4, q, b]
        blk = sh.transpose(3, 0, 2, 1).reshape(B, KP, HWC)  # hw = 4q + m4
        out[:, :, m * HWC:(m + 1) * HWC] = blk
    return out.reshape(B, KP, H, W)


def kernel(x, hash_idx, weights):
    import time

    from concourse.bass_utils import run_bass_kernel_spmd

    in_maps = make_inputs(x, hash_idx, weights)
    last_err = None
    for attempt in range(4):
        try:
            nc = get_nc()
            res = run_bass_kernel_spmd(nc, in_maps, list(range(NCORES)))
            shards = [res.results[m]["out"] for m in range(NCORES)]
            return assemble_output(shards)
        except Exception as e:  # transient NRT/device errors — retry
            last_err = e
            _NC_CACHE.clear()  # rebuild graph/executable on retry
            time.sleep(5.0 * (attempt + 1))
    raise last_err



# revision 23
# speedup vs baseline: 1.0461x; 1.0461x over previous
"""Trainium2 Bass kernel for hash-indexed gather + GEMM (gnn_message_passing).

Reference computation:
    g[b, c, hw, k] = x.reshape(B, C*H*W)[b, hash_idx[c, hw, k]]
    out[kp, b*hw]  = weights[kp, c*k] @ g  (contraction over ck = 288)
    returns [B, KP, H, W]

Strategy (8 NeuronCores, no cross-core communication):
  - Host transposes x to xT[C*H*W, B] float32 so one gathered "row" is the
    value of one (c,pixel) across all 64 images = 256 contiguous bytes.
  - Each core owns 128 of the 1024 output pixels (all images, all channels).
  - On-device dma_gather (SWDGE) gathers rows straight from HBM and writes
    row i to SBUF partition i%128; we order the index list so partitions
    line up with the GEMM contraction dim (ck), i.e. the gather performs the
    im2col transpose for free.
  - 3 ck-chunks (0:128, 128:256, 256:288) accumulate into PSUM. The 32-wide
    third chunk is packed 4x along partitions covering hw%4 classes.
  - Output per core: [KP=64, 8192] = (m4, q, b) column order, reassembled on
    host.
"""

import numpy as np

B, C, H, W = 64, 32, 32, 32
K = 9
KP = 64
HWT = H * W          # 1024
CHW = C * H * W      # 32768
CK = C * K           # 288
NCORES = 8
HWC = HWT // NCORES  # 128 pixels per core
NCOLS = HWC * B      # 8192 output columns per core

_NC_CACHE = {}


def _build_nc():
    """Build the (single-program SPMD) Bass graph; all per-core variation is
    carried by the input data."""
    from concourse import bacc, bass, library_config, mybir, tile

    f32 = mybir.dt.float32
    f32r = mybir.dt.float32r
    i16 = mybir.dt.int16

    nc = bacc.Bacc(num_swdge_queues=4)

    xt = nc.declare_dram_parameter("xt", [CHW, B], f32, isOutput=False)
    idx = nc.declare_dram_parameter("idx", [128, 2304], i16, isOutput=False)
    w = nc.declare_dram_parameter("w", [128, 256], f32, isOutput=False)
    out = nc.declare_dram_parameter("out", [KP, NCOLS], f32, isOutput=True)

    # Load the GPSIMD library before the TileContext preamble so the ~10us
    # IRAM load overlaps the framework's start-of-block barriers.
    nc.gpsimd.load_library(library_config.mlp)

    with tile.TileContext(nc) as tc:
        with (
            tc.tile_pool(name="const", bufs=1) as const_pool,
            tc.tile_pool(name="g", bufs=1) as g_pool,
            tc.tile_pool(name="stage", bufs=1) as stage_pool,
            tc.tile_pool(name="psum", bufs=6, space="PSUM") as psum_pool,
        ):

            idx_sb = const_pool.tile([128, 2304], i16)
            nc.sync.dma_start(out=idx_sb[:], in_=idx[:])
            # float32r tiles: same bits as f32, but lets the fp32r matmuls
            # (1 cycle/row vs 4 for fp32) pass BIR verification.
            w_sb = const_pool.tile([128, 256], f32r)
            nc.sync.dma_start(out=w_sb[:], in_=w[:].bitcast(f32r))

            # Gather targets as per-subcall tiles so Tile's dependency
            # tracking lets supertile s start as soon as ITS slices landed.
            # 2048-idx calls (16 slots / 2 supertiles each) amortize the
            # ~550ns fixed SWDGE cost; the tail calls are 1024-idx so the
            # last round finishes sooner. Schedule: 5 calls per queue
            # (4x2048 + 1x1024 = 9216 rows), one queue per GPSIMD cpu pair.
            # 2048-row calls (16 slots / 2 supertiles each) amortize the
            # ~550ns fixed SWDGE cost; 1024-row tail calls shorten the final
            # round. Per queue: [2048 x4, 1024] = 9216 rows; the four SWDGE
            # queues map to the four GPSIMD cpu pairs and stream ~4-way.
            g0p = [g_pool.tile([128, 16, B], f32r, tag=f"g0p_{t}",
                               name=f"g0p_{t}") for t in range(7)]
            g1p = [g_pool.tile([128, 16, B], f32r, tag=f"g1p_{t}",
                               name=f"g1p_{t}") for t in range(7)]
            g2p = [g_pool.tile([128, 16, B], f32r, tag=f"g2p_{v}",
                               name=f"g2p_{v}") for v in range(2)]
            g0h = [g_pool.tile([128, 8, B], f32r, tag=f"g0h_{h}",
                               name=f"g0h_{h}") for h in range(2)]
            g1h = [g_pool.tile([128, 8, B], f32r, tag=f"g1h_{h}",
                               name=f"g1h_{h}") for h in range(2)]

            nidx_regs = {1024: nc.gpsimd.to_reg(1024),
                         2048: nc.gpsimd.to_reg(2048)}

            def gather(dst, col0, q, nidx=2048):
                nc.gpsimd.dma_gather(
                    dst[:], xt[:].bitcast(f32r),
                    idx_sb[:, col0:col0 + nidx // 16], nidx,
                    nidx_regs[nidx], B, queue_num=q, single_packet=False,
                )

            # Round 1 unblocks supertiles 0..3 (g2 first: shared by all
            # s with matching s%4); rounds 2-4 stream g0/g1 pairs; round 5
            # is the 1024-row tail for supertiles 14/15.
            gather(g2p[0], 2048 + 0, 2)
            gather(g2p[1], 2048 + 128, 3)
            gather(g0p[0], 0, 0)
            gather(g1p[0], 1024, 1)
            for rnd in range(3):
                ta, tb = 1 + 2 * rnd, 2 + 2 * rnd
                gather(g0p[ta], 128 * ta, 0)
                gather(g1p[ta], 1024 + 128 * ta, 1)
                gather(g0p[tb], 128 * tb, 2)
                gather(g1p[tb], 1024 + 128 * tb, 3)
            gather(g0h[0], 896, 0, nidx=1024)
            gather(g1h[0], 1024 + 896, 1, nidx=1024)
            gather(g0h[1], 960, 2, nidx=1024)
            gather(g1h[1], 1024 + 960, 3, nidx=1024)

            stage_t = [stage_pool.tile([KP, 1024], f32, tag=f"st_{r}",
                                       name=f"st_{r}")
                       for r in range(8)]

            def mm(ps, stat, mov, start, stop):
                # float32r: full-rate (1 cycle/row) fp32 matmul on trn2.
                nc.tensor.matmul(ps, stat, mov, start=start, stop=stop)

            def mm_chunk2(ps, m4, g2tile, s0, s1):
                # chunk2: partitions 32*m4..+32 of g2 (AP base must be
                # 0/32/64; the base-96 block runs as K=64 at base 64 with
                # zeroed weight rows for partitions 64..95).
                if m4 < 3:
                    mm(ps, w_sb[32 * m4:32 * m4 + 32, 128:192],
                       g2tile[32 * m4:32 * m4 + 32, s0:s1, :],
                       start=False, stop=True)
                else:
                    mm(ps, w_sb[64:128, 192:256],
                       g2tile[64:128, s0:s1, :], start=False, stop=True)

            # 16 supertiles of 512 output columns; supertile s: m4 = s//4
            # (hw%4 class), q-range 8*(s%4)..+8 (hw//4). s consumes half of
            # gather tile s//2 (supertiles 14/15 have their own 1024-idx
            # tail tiles).
            for s in range(16):
                if s < 14:
                    o = 8 * (s % 2)
                    ga = g0p[s // 2][:, o:o + 8, :]
                    gb = g1p[s // 2][:, o:o + 8, :]
                else:
                    ga = g0h[s - 14][:]
                    gb = g1h[s - 14][:]
                m4, u = s // 4, s % 4
                o2 = 8 * (u % 2)
                ps = psum_pool.tile([KP, 512], f32)
                mm(ps[:], w_sb[:, 0:64], ga, start=True, stop=False)
                mm(ps[:], w_sb[:, 64:128], gb, start=False, stop=False)
                mm_chunk2(ps[:], m4, g2p[u // 2], o2, o2 + 8)
                eng = nc.vector.tensor_copy if s % 2 == 0 else nc.scalar.copy
                eng(out=stage_t[s // 2][:, 512 * (s % 2):512 * (s % 2 + 1)],
                    in_=ps[:])
                if s % 2 == 1:
                    nc.sync.dma_start(
                        out=out[:, 1024 * (s // 2):1024 * (s // 2 + 1)],
                        in_=stage_t[s // 2][:],
                    )

    nc.finalize()
    return nc


def get_nc():
    if "nc" not in _NC_CACHE:
        _NC_CACHE["nc"] = _build_nc()
    return _NC_CACHE["nc"]


def _wrap_idx(flat):
    """SWDGE index layout: unwrapped[i] = wrapped[i % 16, i // 16], replicated
    across the 8 GPSIMD 16-partition groups."""
    n = flat.shape[0]
    wrapped = flat.reshape(n // 16, 16).T  # [16, n/16]
    return np.tile(wrapped, (8, 1))        # [128, n/16]


def make_inputs(x, hash_idx, weights):
    """Host-side sharding/layout prep. Returns in_maps for the 8 cores."""
    x = np.asarray(x, dtype=np.float32)
    hash_idx = np.asarray(hash_idx)
    weights = np.asarray(weights, dtype=np.float32)

    xtr = np.ascontiguousarray(x.reshape(B, CHW).T)  # [CHW, B] f32

    # idxmat[ck, hw] with ck = c*9 + k
    idxmat = (
        hash_idx.reshape(C, HWT, K).transpose(0, 2, 1).reshape(CK, HWT)
    ).astype(np.int16)

    # weights, stationary layout: [contraction partitions, 64 kp]
    w_sb = np.zeros((128, 256), dtype=np.float32)
    w_sb[:, 0:64] = weights[:, 0:128].T
    w_sb[:, 64:128] = weights[:, 128:256].T
    w_sb[:, 128:192] = np.tile(weights[:, 256:288].T, (4, 1))
    # m4=3 special stationary: rows 64..95 (m4=2 data) zero, 96..127 real
    w_sb[96:128, 192:256] = weights[:, 256:288].T

    # column order inside a core: col = m4*2048 + q*64 + b ; hw_local = 4q+m4
    j_arange = np.arange(HWC)
    hw_of_slot = 4 * (j_arange % 32) + j_arange // 32  # slot j -> hw_local

    in_maps = []
    for m in range(NCORES):
        sub = idxmat[:, m * HWC:(m + 1) * HWC]  # [288, 128] int16
        # chunks 0/1: i = j*128 + p (slot-major, ck-local minor)
        c0 = np.ascontiguousarray(sub[0:128, hw_of_slot].T).reshape(-1)
        c1 = np.ascontiguousarray(sub[128:256, hw_of_slot].T).reshape(-1)
        # chunk 2: i = q*128 + m4*32 + ck_local ; partition = 32*m4+ck
        #   value  = idx[256+ck, hw = 4q+m4]
        sub2 = sub[256:288, :]                     # [32, 128]
        arr2 = sub2.T.reshape(32, 4, 32)           # [q, m4, ck]
        c2 = np.ascontiguousarray(arr2).reshape(-1)
        idx_all = np.concatenate(
            [_wrap_idx(c0), _wrap_idx(c1), _wrap_idx(c2)], axis=1
        )
        idx_all = np.ascontiguousarray(idx_all, dtype=np.int16)
        in_maps.append({"xt": xtr, "idx": idx_all, "w": w_sb})
    return in_maps


def assemble_output(shards):
    """shards[m]: [KP, 8192] in (m4, q, b) column order -> [B, KP, H, W]."""
    out = np.empty((B, KP, HWT), dtype=np.float32)
    for m in range(NCORES):
        sh = np.asarray(shards[m]).reshape(KP, 4, 32, B)  # [kp, m4, q, b]
        blk = sh.transpose(3, 0, 2, 1).reshape(B, KP, HWC)  # hw = 4q + m4
        out[:, :, m * HWC:(m + 1) * HWC] = blk
    return out.reshape(B, KP, H, W)


def kernel(x, hash_idx, weights):
    import time

    from concourse.bass_utils import run_bass_kernel_spmd

    in_maps = make_inputs(x, hash_idx, weights)
    last_err = None
    for attempt in range(4):
        try:
            nc = get_nc()
            res = run_bass_kernel_spmd(nc, in_maps, list(range(NCORES)))
            shards = [res.results[m]["out"] for m in range(NCORES)]
            return assemble_output(shards)
        except Exception as e:  # transient NRT/device errors — retry
            last_err = e
            _NC_CACHE.clear()  # rebuild graph/executable on retry
            time.sleep(5.0 * (attempt + 1))
    raise last_err

